# revision 8
# baseline (speedup 1.0000x reference)
"""Trainium2 Bass kernel for nn_Attention (dense transformer attention).

Full module: qkv projection -> per-head softmax(q k^T / sqrt(d)) -> attn @ v
-> output projection (+bias). Returns (out, attn) like the reference.

Distribution: sequence-parallel over 8 NeuronCores. Each core owns a block of
512 query rows and computes, fully on-device:
  - k^T and v for ALL 4096 tokens (replicated compute; cheaper than an
    all-gather at this size, and it removes every collective),
  - q^T for its own 512 rows,
  - per head: dots^T -> exp -> E^T (bf16) feeding O^T = (E @ v)^T on the
    TensorEngine, plus row-major dots -> exp (with row-sum accumulation) ->
    normalized attention rows written straight to HBM,
  - O^T normalized via PE-transposed reciprocal row sums, then the output
    projection for its own rows (bias folded in as a K=1 matmul).

Host side only reshapes/casts inputs (x^T, weights to bf16) and concatenates
the per-core outputs. No FLOPs on the host.
"""

import sys

sys.path.insert(0, "/opt/trn_rl_repo")

import numpy as np
import ml_dtypes

import concourse.bass as bass
import concourse.mybir as mybir
import concourse.tile as tile
from concourse import bacc
from concourse.bass_utils import run_bass_kernel_spmd
from concourse.masks import make_identity

F32 = mybir.dt.float32
BF16 = mybir.dt.bfloat16
EXP = mybir.ActivationFunctionType.Exp
AX_X = mybir.AxisListType.X

N = 4096          # sequence length
D = 512           # model dim
H = 8             # heads
DH = 64           # head dim
NC = 8            # cores
IB = N // NC      # query rows per core = 512
P = 128
SCALE = DH ** -0.5

_nc_cache = {}


def build_nc():
    """Build the single-core Bass program (same program runs SPMD on 8 cores)."""
    nc = bacc.Bacc("TRN2", target_bir_lowering=False, debug=False,
                   enable_asserts=False)

    xT = nc.declare_dram_parameter("xT", [D, N], BF16, isOutput=False)
    xTq = nc.declare_dram_parameter("xTq", [D, IB], BF16, isOutput=False)
    wqkv = nc.declare_dram_parameter("w_qkv", [D, 3 * D], BF16, isOutput=False)
    wout = nc.declare_dram_parameter("w_out", [D, D], BF16, isOutput=False)
    bout = nc.declare_dram_parameter("b_out", [1, D], BF16, isOutput=False)
    attn_o = nc.declare_dram_parameter("attn", [H, IB, N], F32, isOutput=True)
    out_o = nc.declare_dram_parameter("out", [IB, D], F32, isOutput=True)

    KC = D // P        # 4 contraction chunks for the qkv projections
    NJT = N // P       # 32 key tiles
    NIT = IB // P      # 4 query row-tiles per core
    HP = H // 2        # head pairs
    JC2 = N // (2 * D) # 4 row-major 1024-wide chunks

    with tile.TileContext(nc) as tc:
        # ---------- persistent pools ----------
        with (
            tc.tile_pool(name="consts", bufs=1) as consts,
            tc.tile_pool(name="kt2", bufs=1) as kt2_pool,
            tc.tile_pool(name="vsb", bufs=1) as v_pool,
            tc.tile_pool(name="qt2", bufs=1) as qt2_pool,
            tc.tile_pool(name="otp", bufs=1) as ot_pool,
            tc.tile_pool(name="srows", bufs=1) as srow_pool,
        ):
            # W_out stored per head so matmul lhsT/rhs partition bases match
            wout_sb = [consts.tile([DH, D], BF16, name=f"wout{h}", tag=f"wout{h}")
                       for h in range(H)]
            bout_sb = consts.tile([1, D], BF16, name="bout", tag="bout")
            ones_bf = consts.tile([1, P], BF16, name="ones_bf", tag="ones_bf")
            ones_f32 = consts.tile([1, DH], F32, name="ones_f32", tag="ones_f32")
            ident = consts.tile([P, P], F32, name="ident", tag="ident")
            for h in range(H):
                nc.sync.dma_start(wout_sb[h][:], wout[h * DH:(h + 1) * DH, :])
            nc.sync.dma_start(bout_sb[:], bout[:])
            nc.vector.memset(ones_bf[:], 1.0)
            nc.vector.memset(ones_f32[:], 1.0)
            make_identity(nc, ident[:])

            # kT, paired by head: tile hp holds head 2hp on partitions 0-63
            # and head 2hp+1 on partitions 64-127 (enables PE row-group packing)
            kt2 = [kt2_pool.tile([P, N], BF16, name=f"kt2_{hp}", tag=f"kt2_{hp}")
                   for hp in range(HP)]
            v_sb = [v_pool.tile([P, D], BF16, name=f"v{jt}", tag=f"v{jt}")
                    for jt in range(NJT)]
            qt2 = [qt2_pool.tile([P, IB], BF16, name=f"qt2_{hp}", tag=f"qt2_{hp}")
                   for hp in range(HP)]
            # unnormalized O^T per head, and normalized bf16 copies
            ot_raw = [ot_pool.tile([DH, IB], F32, name=f"otr{h}", tag=f"otr{h}")
                      for h in range(H)]
            ot_norm = [ot_pool.tile([DH, IB], BF16, name=f"otn{h}", tag=f"otn{h}")
                       for h in range(H)]
            # per-head reciprocal row sums, column it: [128 rows, 4 row-tiles]
            rs_h = [srow_pool.tile([P, NIT], F32, name=f"rs_{h}", tag=f"rs_{h}")
                    for h in range(H)]

            # ---------- stage 1: qkv projections ----------
            with (
                tc.tile_pool(name="xin", bufs=1) as xin_pool,
                tc.tile_pool(name="ps1", bufs=3, space="PSUM") as ps1,
            ):
                xT_sb = [xin_pool.tile([P, N], BF16, name=f"xT{kc}", tag=f"xT{kc}")
                         for kc in range(KC)]
                xTq_sb = [xin_pool.tile([P, IB], BF16, name=f"xTq{kc}", tag=f"xTq{kc}")
                          for kc in range(KC)]
                wqkv_sb = [xin_pool.tile([P, 3 * D], BF16, name=f"wqkv{kc}", tag=f"wqkv{kc}")
                           for kc in range(KC)]
                for kc in range(KC):
                    nc.sync.dma_start(xT_sb[kc][:], xT[kc * P:(kc + 1) * P, :])
                    nc.sync.dma_start(xTq_sb[kc][:], xTq[kc * P:(kc + 1) * P, :])
                    nc.sync.dma_start(wqkv_sb[kc][:], wqkv[kc * P:(kc + 1) * P, :])

                # k^T: [512 feat, 4096 tok] -> paired-head tiles
                for ft in range(KC):
                    for jc in range(N // D):
                        ps = ps1.tile([P, D], F32, name="ps1", tag="ps1")
                        for kc in range(KC):
                            nc.tensor.matmul(
                                ps[:], wqkv_sb[kc][:, D + ft * P: D + (ft + 1) * P],
                                xT_sb[kc][:, jc * D:(jc + 1) * D],
                                start=(kc == 0), stop=(kc == KC - 1))
                        nc.vector.tensor_copy(kt2[ft][:, jc * D:(jc + 1) * D], ps[:])

                # v: token-major [4096, 512]
                for jt in range(NJT):
                    ps = ps1.tile([P, D], F32, name="ps1", tag="ps1")
                    for kc in range(KC):
                        nc.tensor.matmul(
                            ps[:], xT_sb[kc][:, jt * P:(jt + 1) * P],
                            wqkv_sb[kc][:, 2 * D:3 * D],
                            start=(kc == 0), stop=(kc == KC - 1))
                    nc.vector.tensor_copy(v_sb[jt][:], ps[:])

                # q^T for own rows -> paired-head tiles
                for ft in range(KC):
                    ps = ps1.tile([P, IB], F32, name="ps1", tag="ps1")
                    for kc in range(KC):
                        nc.tensor.matmul(
                            ps[:], wqkv_sb[kc][:, ft * P:(ft + 1) * P],
                            xTq_sb[kc][:],
                            start=(kc == 0), stop=(kc == KC - 1))
                    nc.vector.tensor_copy(qt2[ft][:], ps[:])

            # ---------- stage 2: attention ----------
            with (
                tc.tile_pool(name="psAB", bufs=3, space="PSUM") as psAB,
                tc.tile_pool(name="psO", bufs=2, space="PSUM") as psO,
                tc.tile_pool(name="et", bufs=3) as et_pool,
                tc.tile_pool(name="erow", bufs=4) as erow_pool,
                tc.tile_pool(name="sacc", bufs=4) as sacc_pool,
            ):
                for hp in range(HP):
                    h0, h1 = 2 * hp, 2 * hp + 1
                    # --- phase A: dots^T -> E^T (bf16) -> O^T accumulation
                    ops0 = psO.tile([DH, IB], F32, name="opsum", tag="opsum")
                    ops1 = psO.tile([DH, IB], F32, name="opsum", tag="opsum")
                    for jt in range(NJT):
                        dps = psAB.tile([P, 2 * IB], F32, name="psAB", tag="psAB")
                        # the two heads sit in different PE row groups -> concurrent
                        nc.tensor.matmul(
                            dps[:, 0:IB], kt2[hp][0:DH, jt * P:(jt + 1) * P],
                            qt2[hp][0:DH, :], start=True, stop=True)
                        nc.tensor.matmul(
                            dps[:, IB:2 * IB], kt2[hp][DH:P, jt * P:(jt + 1) * P],
                            qt2[hp][DH:P, :], start=True, stop=True)
                        et = et_pool.tile([P, 2 * IB], BF16, name="et", tag="et")
                        nc.scalar.activation(et[:], dps[:], EXP, scale=SCALE)
                        nc.tensor.matmul(ops0[:], v_sb[jt][:, h0 * DH:(h0 + 1) * DH],
                                         et[:, 0:IB],
                                         start=(jt == 0), stop=(jt == NJT - 1))
                        nc.tensor.matmul(ops1[:], v_sb[jt][:, h1 * DH:(h1 + 1) * DH],
                                         et[:, IB:2 * IB],
                                         start=(jt == 0), stop=(jt == NJT - 1))
                    nc.vector.tensor_copy(ot_raw[h0][:], ops0[:])
                    nc.vector.tensor_copy(ot_raw[h1][:], ops1[:])

                    # --- phase B: row-major dots -> normalized attn rows -> HBM
                    for h, pb in ((h0, 0), (h1, DH)):
                        for it in range(NIT):
                            erow = erow_pool.tile([P, N], F32, name="erow", tag="erow")
                            sacc = sacc_pool.tile([P, JC2], F32, name="sacc", tag="sacc")
                            for jc2 in range(JC2):
                                rps = psAB.tile([P, 2 * D], F32, name="psAB", tag="psAB")
                                for half in range(2):
                                    jcs = (2 * jc2 + half) * D
                                    nc.tensor.matmul(
                                        rps[:, half * D:(half + 1) * D],
                                        qt2[hp][pb:pb + DH, it * P:(it + 1) * P],
                                        kt2[hp][pb:pb + DH, jcs:jcs + D],
                                        start=True, stop=True)
                                nc.scalar.activation(
                                    erow[:, jc2 * 2 * D:(jc2 + 1) * 2 * D], rps[:],
                                    EXP, scale=SCALE,
                                    accum_out=sacc[:, jc2:jc2 + 1])
                            stot = sacc_pool.tile([P, 1], F32, name="stot", tag="stot")
                            nc.vector.reduce_sum(stot[:], sacc[:], axis=AX_X)
                            nc.vector.reciprocal(rs_h[h][:, it:it + 1], stot[:])
                            nc.vector.tensor_scalar_mul(erow[:], erow[:],
                                                        rs_h[h][:, it:it + 1])
                            nc.sync.dma_start(
                                attn_o[h, it * P:(it + 1) * P, :], erow[:])

            # ---------- stage 3: normalize O^T and output projection ----------
            with (
                tc.tile_pool(name="psP", bufs=2, space="PSUM") as psP,
                tc.tile_pool(name="psBc", bufs=2, space="PSUM") as psBc,
                tc.tile_pool(name="bcast", bufs=2) as bc_pool,
                tc.tile_pool(name="osb", bufs=2) as out_pool,
            ):
                for h in range(H):
                    # transpose each [128,1] reciprocal-sum column -> [1,128]
                    rs_row = bc_pool.tile([1, IB], F32, name="rs_row", tag="rs_row")
                    for it in range(NIT):
                        tps = psBc.tile([1, P], F32, name="tps", tag="tps")
                        nc.tensor.transpose(tps[:], rs_h[h][:, it:it + 1], ident[:])
                        nc.vector.tensor_copy(rs_row[:, it * P:(it + 1) * P], tps[:])
                    bps = psBc.tile([DH, IB], F32, name="bps", tag="bps")
                    nc.tensor.matmul(bps[:], ones_f32[:], rs_row[:],
                                     start=True, stop=True)
                    bsb = bc_pool.tile([DH, IB], F32, name="bsb", tag="bsb")
                    nc.vector.tensor_copy(bsb[:], bps[:])
                    nc.vector.tensor_mul(ot_norm[h][:], ot_raw[h][:], bsb[:])
                for it in range(NIT):
                    pps = psP.tile([P, D], F32, name="pps", tag="pps")
                    for h in range(H):
                        nc.tensor.matmul(pps[:], ot_norm[h][:, it * P:(it + 1) * P],
                                         wout_sb[h][:], start=(h == 0), stop=False)
                    nc.tensor.matmul(pps[:], ones_bf[:], bout_sb[:],
                                     start=False, stop=True)
                    osb = out_pool.tile([P, D], F32, name="osb", tag="osb")
                    nc.vector.tensor_copy(osb[:], pps[:])
                    nc.sync.dma_start(out_o[it * P:(it + 1) * P, :], osb[:])

    nc.compile()
    return nc


def _get_nc():
    if "nc" not in _nc_cache:
        _nc_cache["nc"] = build_nc()
    return _nc_cache["nc"]


def _prep_in_maps(x, W_qkv, W_out, b_out):
    bf = ml_dtypes.bfloat16
    xT = np.ascontiguousarray(np.asarray(x, np.float32)[0].T).astype(bf)  # [512, 4096]
    wqkv = np.ascontiguousarray(np.asarray(W_qkv, np.float32)).astype(bf)
    wout = np.ascontiguousarray(np.asarray(W_out, np.float32)).astype(bf)
    bo = np.asarray(b_out, np.float32).reshape(1, D).astype(bf)
    in_maps = []
    for c in range(NC):
        xTq = np.ascontiguousarray(xT[:, c * IB:(c + 1) * IB])
        in_maps.append({"xT": xT, "xTq": xTq, "w_qkv": wqkv,
                        "w_out": wout, "b_out": bo})
    return in_maps


def run(inputs, trace=False, tmpdir=None):
    nc = _get_nc()
    in_maps = _prep_in_maps(inputs["x"], inputs["W_qkv"], inputs["W_out"],
                            inputs["b_out"])
    res = run_bass_kernel_spmd(nc, in_maps, core_ids=list(range(NC)),
                               trace=trace, tmpdir=tmpdir)
    outs = [res.results[c]["out"] for c in range(NC)]
    attns = [res.results[c]["attn"] for c in range(NC)]
    out_full = np.concatenate(outs, axis=0)[None]                   # [1, 4096, 512]
    attn_full = np.concatenate(attns, axis=1)[None]                 # [1, 8, 4096, 4096]
    return (out_full.astype(np.float32), attn_full.astype(np.float32)), res


def kernel(**inputs):
    (out_full, attn_full), _ = run(inputs, trace=False)
    return out_full, attn_full
